# revision 6
# baseline (speedup 1.0000x reference)
"""Trainium2 Bass kernel for CausalCrossAttention (B=8, T=769, C=1024, H=16).

Sharding: data-parallel over batch B=8 across the 8 NeuronCores (one batch
element per core, SPMD).

v1 (bf16 rewrite of the fp32r baseline):
  - All matmul operands bf16 (host-cast); PSUM accumulates fp32. Halves HBM
    traffic (the baseline lost ~80us waiting on fp32 weight DMAs) and lets a
    single matmul stream the full 770-col T range (bf16 moving max = 1024).
  - Q/K projections in [c_out, t] layout with partial rotary via a host
    permutation (even/odd pair split per head) + partition-block-swap DMAs +
    3 DVE ops (bf16 = 2x DVE rate).
  - S^T attention row-tiled: head pair (2j, 2j+1) computed CONCURRENTLY by two
    K=64 matmuls in disjoint PE row groups (tile_position auto from
    base_partition 0 / 64) -> S^T costs N cycles per head PAIR, and the
    baseline's qz sibling-zeroing copies disappear.
  - Per (j, nk): both heads' S^T go to one 4-bank psum tile [128, 2, 1024]
    (bank-disjoint halves), ONE 3D-AP exp covers both heads (halves ACT
    instruction overhead; ACT is the attention-phase bottleneck).
  - PV with M=66 per head (64 v-dims + ones column for the softmax
    denominator + 1 pad col for evenness), accumulated in [128, 2, 1024].
  - Softmax division: denominator rows DMA'd from PSUM with partition
    broadcast, one approx reciprocal per head pair, two DVE mults.
  - Output projection in [c_out, t] layout (host transposes back).
"""

import os

import numpy as np

B, T, C = 8, 769, 1024
H, HD, L = 16, 64, 32
COND = 256
NCI = 8
NCO = 8
TP = 770
HW = HD + 2  # per-head vaug block: 64 v dims + ones col + pad col
VW = H * HW  # 1056

_CACHE = {}

# Per-(kv-tile) q ranges + mask offset: kv tile nk covers cols
# [128nk, 128nk+128); allowed iff kv_col < 256 + q_col.
R0SUB = {0: (0, None), 1: (0, None), 2: (0, 0), 3: (128, 128),
         4: (256, 256), 5: (384, 384)}


def _build_program():
    import concourse.mybir as mybir
    import concourse.tile as tile
    from concourse import bacc

    f32 = mybir.dt.float32
    bf16 = mybir.dt.bfloat16
    Exp = mybir.ActivationFunctionType.Exp
    Ident = mybir.ActivationFunctionType.Identity

    nc = bacc.Bacc("TRN2", target_bir_lowering=False)

    xqT_d = nc.dram_tensor("xqT", [C, TP], bf16, kind="ExternalInput")
    xkvT_d = nc.dram_tensor("xkvT", [C, TP], bf16, kind="ExternalInput")
    wq_d = nc.dram_tensor("wqT", [C, C], bf16, kind="ExternalInput")
    wk_d = nc.dram_tensor("wkT", [C, C], bf16, kind="ExternalInput")
    wv_d = nc.dram_tensor("wvT", [C, C], bf16, kind="ExternalInput")
    wp_d = nc.dram_tensor("wpT", [C, C], bf16, kind="ExternalInput")
    bq_d = nc.dram_tensor("bq2", [128, NCO], f32, kind="ExternalInput")
    bk_d = nc.dram_tensor("bk2", [128, NCO], f32, kind="ExternalInput")
    bp_d = nc.dram_tensor("bp2", [128, NCO], f32, kind="ExternalInput")
    bv_d = nc.dram_tensor("bv1", [1, C], f32, kind="ExternalInput")
    cos_d = nc.dram_tensor("cosP", [128, TP], bf16, kind="ExternalInput")
    sin_d = nc.dram_tensor("sinP", [128, TP], bf16, kind="ExternalInput")
    m0_d = nc.dram_tensor("m0", [128, 128], bf16, kind="ExternalInput")
    out_d = nc.dram_tensor("outT", [C, TP], bf16, kind="ExternalOutput")

    with tile.TileContext(nc) as tc:
        with (
            tc.tile_pool(name="consts", bufs=1) as consts,
            tc.tile_pool(name="wpool", bufs=1) as wpool,
            tc.tile_pool(name="qkpool", bufs=1) as qkpool,
            tc.tile_pool(name="vpool", bufs=1) as vpool,
        ):
            cos_sb = consts.tile([128, TP], bf16, tag="cos")
            sin_sb = consts.tile([128, TP], bf16, tag="sin")
            m0_sb = consts.tile([128, 128], bf16, tag="m0")
            bq_sb = consts.tile([128, NCO], f32, tag="bq")
            bk_sb = consts.tile([128, NCO], f32, tag="bk")
            bp_sb = consts.tile([128, NCO], f32, tag="bp")
            ones16 = consts.tile([128, 16], f32, tag="ones16")
            nc.vector.memset(ones16, 1.0)
            zcol = consts.tile([128, 32], f32, tag="zcol")
            nc.vector.memset(zcol, 0.0)
            nc.scalar.dma_start(out=cos_sb, in_=cos_d[:, :])
            nc.scalar.dma_start(out=sin_sb, in_=sin_d[:, :])
            nc.scalar.dma_start(out=m0_sb, in_=m0_d[:, :])
            nc.scalar.dma_start(out=bq_sb, in_=bq_d[:, :])
            nc.scalar.dma_start(out=bk_sb, in_=bk_d[:, :])
            nc.scalar.dma_start(out=bp_sb, in_=bp_d[:, :])

            qT = qkpool.tile([128, NCI, TP], bf16, tag="qT")
            kT = qkpool.tile([128, NCI, TP], bf16, tag="kT")
            vaug = vpool.tile([128, 7, VW], bf16, tag="vaug")
            yT = None

            def load_w(wdram, pfx, q):
                ws = []
                for ci in range(NCI):
                    wt = wpool.tile([128, C], bf16, tag=f"{pfx}{ci}",
                                    name=f"{pfx}{ci}")
                    q.dma_start(
                        out=wt, in_=wdram[ci * 128:(ci + 1) * 128, :])
                    ws.append(wt)
                return ws

            def proj_qk(w, x, b_sb, outT, psA, shpool):
                for cog in (range(0, 4), range(4, 8)):
                    pss = {}
                    for co in cog:
                        pss[co] = psA.tile([128, 1024], f32, tag="ps",
                                           name=f"psqk{co}")
                    for ci in range(NCI):
                        for co in cog:
                            for (lo, hi) in ((0, 512), (512, TP)):
                                nc.tensor.matmul(
                                    pss[co][:, lo:hi],
                                    w[ci][:, co * 128:(co + 1) * 128],
                                    x[:, ci, lo:hi],
                                    start=(ci == 0), stop=(ci == NCI - 1))
                    for co in cog:
                        proj_qk_tail(pss[co], co, b_sb, outT, shpool)

            def proj_qk_tail(ps, co, b_sb, outT, shpool):
                # bias add + PSUM->SBUF(bf16) on ACT
                nc.scalar.activation(
                    out=outT[:, co, :], in_=ps[:, 0:TP], func=Ident,
                    bias=b_sb[:, co:co + 1], scale=1.0)
                # rotary: swap 16-row blocks of the (host-permuted) rotary
                # dims, then q = q*cos + swapped*sin (cos rows outside the
                # rotary dims are 1.0, sin rows are 0.0).
                sh = shpool.tile([128, TP], bf16, tag="sh", name="sh")
                nc.sync.dma_start(
                    out=sh[32:64, :], in_=outT[32:64, co, :])
                for s in (0, 64):
                    nc.sync.dma_start(
                        out=sh[s:s + 16, :], in_=outT[s + 16:s + 32, co, :])
                    nc.sync.dma_start(
                        out=sh[s + 16:s + 32, :], in_=outT[s:s + 16, co, :])
                nc.vector.tensor_mul(
                    sh[0:96, :], sh[0:96, :], sin_sb[0:96, :])
                nc.vector.tensor_mul(
                    outT[:, co, :], outT[:, co, :], cos_sb)
                nc.vector.tensor_add(
                    outT[0:96, co, :], outT[0:96, co, :], sh[0:96, :])

            def proj_v(w, x, psA, bv_sb):
                for g in (range(0, 4), range(4, 7)):
                    pss = {}
                    for tt in g:
                        pss[tt] = psA.tile([128, 1024], f32, tag="ps",
                                           name=f"psv{tt}")
                    for ci in range(NCI):
                        for tt in g:
                            tsz = 128 if tt < 6 else 1
                            for hf in (0, 1):
                                nc.tensor.matmul(
                                    pss[tt][:tsz, hf * 512:hf * 512 + 512],
                                    x[:, ci, tt * 128:tt * 128 + tsz],
                                    w[ci][:, hf * 512:hf * 512 + 512],
                                    start=(ci == 0), stop=(ci == NCI - 1))
                    for tt in g:
                        tsz = 128 if tt < 6 else 1
                        va = vaug[:tsz, tt, :].rearrange(
                            "p (h e) -> p h e", e=HW)
                        nc.vector.tensor_add(
                            va[:, :, 0:HD],
                            pss[tt][:tsz, :].rearrange("p (h d) -> p h d", h=H),
                            bv_sb[:tsz, :].rearrange("p (h d) -> p h d", h=H))
                        nc.vector.tensor_copy(
                            va[:, :, HD:HD + 1], ones16[:tsz, :].unsqueeze(2))
                        nc.vector.tensor_copy(
                            va[:, :, HD + 1:HW], zcol[:tsz, 0:16].unsqueeze(2))

            def attn_j(j, psS, psO, pt_pool, rdbc_pool, stg_pool, dnd):
                """Head pair (2j, 2j+1): row-tiled S^T, fused exp, PV."""
                stAB = psS.tile([128, 2, 1024], f32, tag="st", name=f"st{j}")
                ovAB = psO.tile([128, 2, 1024], f32, tag="ov", name=f"ov{j}")
                pt = pt_pool.tile([128, 2, 1024], bf16, tag="pt",
                                  name=f"pt{j}")
                for nk in range(6):
                    qlo, moff = R0SUB[nk]
                    ks = slice(nk * 128, (nk + 1) * 128)
                    # two concurrent K=64 matmuls in disjoint PE row groups
                    for (lo, hi) in ((qlo, 512), (512, TP)):
                        nc.tensor.matmul(
                            stAB[:, 0, lo:hi], kT[0:64, j, ks],
                            qT[0:64, j, lo:hi], start=True, stop=True)
                        nc.tensor.matmul(
                            stAB[:, 1, lo:hi], kT[64:128, j, ks],
                            qT[64:128, j, lo:hi], start=True, stop=True)
                    # one exp over both heads (3D AP, bank-disjoint halves)
                    nc.scalar.activation(
                        out=pt[:, :, qlo:TP], in_=stAB[:, :, qlo:TP],
                        func=Exp, scale=0.125)
                    if moff is not None:
                        nc.gpsimd.tensor_mul(
                            pt[:, 0, moff:moff + 128],
                            pt[:, 0, moff:moff + 128], m0_sb)
                        nc.gpsimd.tensor_mul(
                            pt[:, 1, moff:moff + 128],
                            pt[:, 1, moff:moff + 128], m0_sb)
                    for h in (0, 1):
                        vs = slice((2 * j + h) * HW, (2 * j + h) * HW + HW)
                        for (lo, hi) in ((qlo, 512), (512, TP)):
                            nc.tensor.matmul(
                                ovAB[0:HW, h, lo:hi], vaug[:, nk, vs],
                                pt[:, h, lo:hi],
                                start=(nk == 0), stop=False)
                # kv col 768 (single kv row): q cols 512:770, col 512 masked.
                nc.tensor.matmul(
                    stAB[0:1, 0, 512:TP], kT[0:64, j, 768:769],
                    qT[0:64, j, 512:TP], start=True, stop=True)
                nc.tensor.matmul(
                    stAB[0:1, 1, 512:TP], kT[64:128, j, 768:769],
                    qT[64:128, j, 512:TP], start=True, stop=True)
                nc.scalar.activation(
                    out=pt[0:1, :, 512:TP], in_=stAB[0:1, :, 512:TP],
                    func=Exp, scale=0.125)
                nc.gpsimd.tensor_copy(
                    pt[0:1, :, 512:513], zcol[0:1, 0:2].unsqueeze(2))
                for h in (0, 1):
                    vs = slice((2 * j + h) * HW, (2 * j + h) * HW + HW)
                    nc.tensor.matmul(
                        ovAB[0:HW, h, 512:TP], vaug[0:1, 6, vs],
                        pt[0:1, h, 512:TP], start=False, stop=True)

                # softmax division: denominator rows (PSUM row 64 of each
                # head) broadcast into rdbc via staging, one reciprocal,
                # two DVE mults PSUM->SBUF(bf16).
                stg = stg_pool.tile([1, 2, 1024], f32, tag="stg", bufs=2,
                                    name=f"stg{j}")
                nc.vector.tensor_copy(
                    stg[0:1, :, 0:TP], ovAB[HD:HD + 1, :, 0:TP])
                nc.sync.dma_start(
                    out=dnd[j:j + 1, :, :], in_=stg[0:1, :, 0:TP])
                rdbc = rdbc_pool.tile([128, TP], f32, tag="rdbc",
                                      name=f"rdbc{j}")
                nc.gpsimd.dma_start(
                    out=rdbc[0:64, :],
                    in_=dnd[j:j + 1, 0, :].broadcast_to((64, TP)))
                nc.gpsimd.dma_start(
                    out=rdbc[64:128, :],
                    in_=dnd[j:j + 1, 1, :].broadcast_to((64, TP)))
                nc.vector.reciprocal_approx_fast(out=rdbc, in_=rdbc)
                nc.vector.tensor_mul(
                    yT[0:64, j, :], ovAB[0:HD, 0, 0:TP], rdbc[0:64, :])
                nc.vector.tensor_mul(
                    yT[64:128, j, :], ovAB[0:HD, 1, 0:TP], rdbc[64:128, :])

            def proj_out(w, psB, opool):
                for cog in (range(0, 4), range(4, 8)):
                    pss = {}
                    for co in cog:
                        pss[co] = psB.tile([128, 1024], f32, tag="pso",
                                           name=f"pso{co}")
                    for ci in range(NCI):
                        for co in cog:
                            for (lo, hi) in ((0, 512), (512, TP)):
                                nc.tensor.matmul(
                                    pss[co][:, lo:hi],
                                    w[ci][:, co * 128:(co + 1) * 128],
                                    yT[:, ci, lo:hi],
                                    start=(ci == 0), stop=(ci == NCI - 1))
                    for co in cog:
                        ot = opool.tile([128, TP], bf16, tag="ot", name="ot")
                        nc.scalar.activation(
                            out=ot, in_=pss[co][:, 0:TP], func=Ident,
                            bias=bp_sb[:, co:co + 1], scale=1.0)
                        nc.sync.dma_start(
                            out=out_d[co * 128:(co + 1) * 128, :], in_=ot)

            # ---- phase 1: projections ----
            with (
                tc.tile_pool(name="psA", bufs=4, space="PSUM") as psA,
                tc.tile_pool(name="xq", bufs=1) as xqp,
                tc.tile_pool(name="xkv", bufs=1) as xkp,
                tc.tile_pool(name="shpool", bufs=2) as shpool,
            ):
                bv_sb = xqp.tile([128, C], f32, tag="bv")
                nc.gpsimd.dma_start(
                    out=bv_sb, in_=bv_d[0:1, :].broadcast_to((128, C)))
                xq = xqp.tile([128, NCI, TP], bf16, tag="xq")
                xkv = xkp.tile([128, NCI, TP], bf16, tag="xkv")
                # sync queue: xq+wq interleaved per-ci so MMs start early;
                # scalar queue: xkv+wk in parallel; tensor queue: wv, wp.
                wq, wk = [], []
                for ci in range(NCI):
                    nc.sync.dma_start(
                        out=xq[:, ci, :],
                        in_=xqT_d[ci * 128:(ci + 1) * 128, :])
                    wt = wpool.tile([128, C], bf16, tag=f"wq{ci}",
                                    name=f"wq{ci}")
                    nc.sync.dma_start(
                        out=wt, in_=wq_d[ci * 128:(ci + 1) * 128, :])
                    wq.append(wt)
                    nc.scalar.dma_start(
                        out=xkv[:, ci, :],
                        in_=xkvT_d[ci * 128:(ci + 1) * 128, :])
                wk = load_w(wk_d, "wk", nc.scalar)
                wv = load_w(wv_d, "wv", nc.gpsimd)
                proj_qk(wq, xq, bq_sb, qT, psA, shpool)
                proj_qk(wk, xkv, bk_sb, kT, psA, shpool)
                proj_v(wv, xkv, psA, bv_sb)

            # ---- phases 2+3 share yT ----
            with tc.tile_pool(name="ypool", bufs=1) as ypool:
                yT = ypool.tile([128, NCI, TP], bf16, tag="yT")
                # ---- phase 2: attention ----
                with (
                    tc.tile_pool(name="psS", bufs=1, space="PSUM") as psS,
                    tc.tile_pool(name="psO", bufs=1, space="PSUM") as psO,
                    tc.tile_pool(name="ptp", bufs=2) as pt_pool,
                    tc.tile_pool(name="rdbcp", bufs=2) as rdbc_pool,
                    tc.tile_pool(name="stgp", bufs=2) as stg_pool,
                    tc.tile_pool(name="rddp", bufs=1, space="DRAM") as dram_pool,
                ):
                    wp = load_w(wp_d, "wp", nc.gpsimd)  # prefetch
                    dnd = dram_pool.tile([NCI, 2, TP], f32, tag="dnd")
                    for j in range(NCI):
                        attn_j(j, psS, psO, pt_pool, rdbc_pool, stg_pool, dnd)

                # ---- phase 3: output projection ----
                with (
                    tc.tile_pool(name="psB", bufs=4, space="PSUM") as psB,
                    tc.tile_pool(name="opool", bufs=3) as opool,
                ):
                    proj_out(wp, psB, opool)

    nc.compile()
    return nc


def _host_prep(x_q, x_kv, rotary_pos_emb, Wq, bq, Wk, bk, Wv, bv, Wp, bp):
    f = np.float32
    x_q = np.asarray(x_q, f)
    x_kv = np.asarray(x_kv, f)
    freqs = np.asarray(rotary_pos_emb, f)

    # Even/odd pair-split permutation of the first 32 dims of each head, so
    # rotate_half becomes a 16-partition block swap on chip.
    perm = np.arange(C)
    for h in range(H):
        b0 = h * HD
        blk = np.empty(HD, np.int64)
        blk[0:16] = b0 + np.arange(0, 32, 2)
        blk[16:32] = b0 + np.arange(1, 32, 2)
        blk[32:64] = b0 + np.arange(32, 64)
        perm[b0:b0 + HD] = blk

    def wT(W, p=None):
        W = np.asarray(W, f)
        if p is not None:
            W = W[p, :]
        return np.ascontiguousarray(W.T).astype(np.float32)

    cosE = np.cos(freqs[:, 0::2]).T
    cosO = np.cos(freqs[:, 1::2]).T
    sinE = -np.sin(freqs[:, 0::2]).T
    sinO = np.sin(freqs[:, 1::2]).T
    cosP = np.ones((128, TP), f)
    sinP = np.zeros((128, TP), f)
    for s in (0, 64):
        cosP[s:s + 16, :T] = cosE
        cosP[s + 16:s + 32, :T] = cosO
        sinP[s:s + 16, :T] = sinE
        sinP[s + 16:s + 32, :T] = sinO

    p_idx = np.arange(128)[:, None]
    f_idx = np.arange(128)[None, :]
    m0 = (p_idx < f_idx).astype(f)

    import ml_dtypes
    bf = ml_dtypes.bfloat16

    bqp = np.asarray(bq, f)[perm]
    bkp = np.asarray(bk, f)[perm]
    shared = {
        "wqT": wT(Wq, perm).astype(bf),
        "wkT": wT(Wk, perm).astype(bf),
        "wvT": wT(Wv).astype(bf),
        "wpT": wT(Wp).astype(bf),
        "bq2": np.ascontiguousarray(bqp.reshape(NCO, 128).T).astype(f),
        "bk2": np.ascontiguousarray(bkp.reshape(NCO, 128).T).astype(f),
        "bp2": np.ascontiguousarray(
            np.asarray(bp, f).reshape(NCO, 128).T).astype(f),
        "bv1": np.asarray(bv, f).reshape(1, C).copy(),
        "cosP": np.ascontiguousarray(cosP).astype(bf),
        "sinP": np.ascontiguousarray(sinP).astype(bf),
        "m0": np.ascontiguousarray(m0).astype(bf),
    }

    def padT(xt):
        out = np.zeros((C, TP), np.float32)
        out[:, :T] = xt
        return out.astype(bf)

    in_maps = []
    for b in range(B):
        m = dict(shared)
        m["xqT"] = padT(x_q[b].T)
        m["xkvT"] = padT(x_kv[b].T)
        in_maps.append(m)
    return in_maps


def kernel(x_q, x_kv, rotary_pos_emb, Wq, bq, Wk, bk, Wv, bv, Wp, bp):
    from concourse.bass_utils import run_bass_kernel_spmd

    if "nc" not in _CACHE:
        _CACHE["nc"] = _build_program()
    nc = _CACHE["nc"]

    in_maps = _host_prep(x_q, x_kv, rotary_pos_emb,
                         Wq, bq, Wk, bk, Wv, bv, Wp, bp)
    trace = os.environ.get("BTK_TRACE", "0") == "1"
    res = run_bass_kernel_spmd(
        nc, in_maps, core_ids=list(range(B)), trace=trace)
    _CACHE["last_result"] = res
    out = np.stack(
        [np.asarray(r["outT"][:, :T], np.float32).T for r in res.results],
        axis=0)
    return out


# revision 10
# speedup vs baseline: 1.3814x; 1.3814x over previous
"""Trainium2 Bass kernel for CausalCrossAttention (B=8, T=769, C=1024, H=16).

Sharding: data-parallel over batch B=8 across the 8 NeuronCores (one batch
element per core, SPMD).

v1 (bf16 rewrite of the fp32r baseline):
  - All matmul operands bf16 (host-cast); PSUM accumulates fp32. Halves HBM
    traffic (the baseline lost ~80us waiting on fp32 weight DMAs) and lets a
    single matmul stream the full 770-col T range (bf16 moving max = 1024).
  - Q/K projections in [c_out, t] layout with partial rotary via a host
    permutation (even/odd pair split per head) + partition-block-swap DMAs +
    3 DVE ops (bf16 = 2x DVE rate).
  - S^T attention row-tiled: head pair (2j, 2j+1) computed CONCURRENTLY by two
    K=64 matmuls in disjoint PE row groups (tile_position auto from
    base_partition 0 / 64) -> S^T costs N cycles per head PAIR, and the
    baseline's qz sibling-zeroing copies disappear.
  - Per (j, nk): both heads' S^T go to one 4-bank psum tile [128, 2, 1024]
    (bank-disjoint halves), ONE 3D-AP exp covers both heads (halves ACT
    instruction overhead; ACT is the attention-phase bottleneck).
  - PV with M=66 per head (64 v-dims + ones column for the softmax
    denominator + 1 pad col for evenness), accumulated in [128, 2, 1024].
  - Softmax division: denominator rows DMA'd from PSUM with partition
    broadcast, one approx reciprocal per head pair, two DVE mults.
  - Output projection in [c_out, t] layout (host transposes back).
"""

import os

import numpy as np

B, T, C = 8, 769, 1024
H, HD, L = 16, 64, 32
COND = 256
NCI = 8
NCO = 8
TP = 770
HW = HD + 2  # per-head vaug block: 64 v dims + ones col + pad col
VW = H * HW  # 1056

_CACHE = {}

# Per-(kv-tile) q ranges + mask offset: kv tile nk covers cols
# [128nk, 128nk+128); allowed iff kv_col < 256 + q_col.
R0SUB = {0: (0, None), 1: (0, None), 2: (0, 0), 3: (128, 128),
         4: (256, 256), 5: (384, 384)}


def _build_program():
    import concourse.mybir as mybir
    import concourse.tile as tile
    from concourse import bacc

    f32 = mybir.dt.float32
    bf16 = mybir.dt.bfloat16
    Exp = mybir.ActivationFunctionType.Exp
    Ident = mybir.ActivationFunctionType.Identity

    nc = bacc.Bacc("TRN2", target_bir_lowering=False)

    xqT_d = nc.dram_tensor("xqT", [C, TP], bf16, kind="ExternalInput")
    xkvT_d = nc.dram_tensor("xkvT", [C, TP], bf16, kind="ExternalInput")
    wq_d = nc.dram_tensor("wqT", [C, C], bf16, kind="ExternalInput")
    wk_d = nc.dram_tensor("wkT", [C, C], bf16, kind="ExternalInput")
    wv_d = nc.dram_tensor("wvT", [C, C], bf16, kind="ExternalInput")
    wp_d = nc.dram_tensor("wpT", [C, C], bf16, kind="ExternalInput")
    bq_d = nc.dram_tensor("bq2", [128, NCO], f32, kind="ExternalInput")
    bk_d = nc.dram_tensor("bk2", [128, NCO], f32, kind="ExternalInput")
    bp_d = nc.dram_tensor("bp2", [128, NCO], f32, kind="ExternalInput")
    bv_d = nc.dram_tensor("bv1", [1, C], f32, kind="ExternalInput")
    cos_d = nc.dram_tensor("cosP", [128, TP], bf16, kind="ExternalInput")
    sin_d = nc.dram_tensor("sinP", [128, TP], bf16, kind="ExternalInput")
    m0_d = nc.dram_tensor("m0", [128, 128], bf16, kind="ExternalInput")
    out_d = nc.dram_tensor("outT", [C, TP], bf16, kind="ExternalOutput")

    with tile.TileContext(nc) as tc:
        with (
            tc.tile_pool(name="consts", bufs=1) as consts,
            tc.tile_pool(name="wpool", bufs=1) as wpool,
            tc.tile_pool(name="qkpool", bufs=1) as qkpool,
            tc.tile_pool(name="vpool", bufs=1) as vpool,
        ):
            cos_sb = consts.tile([128, TP], bf16, tag="cos")
            sin_sb = consts.tile([128, TP], bf16, tag="sin")
            m0_sb = consts.tile([128, 128], bf16, tag="m0")
            bq_sb = consts.tile([128, NCO], f32, tag="bq")
            bk_sb = consts.tile([128, NCO], f32, tag="bk")
            bp_sb = consts.tile([128, NCO], f32, tag="bp")
            ones16 = consts.tile([128, 16], f32, tag="ones16")
            nc.vector.memset(ones16, 1.0)
            zcol = consts.tile([128, 32], f32, tag="zcol")
            nc.vector.memset(zcol, 0.0)
            nc.scalar.dma_start(out=cos_sb, in_=cos_d[:, :])
            nc.scalar.dma_start(out=sin_sb, in_=sin_d[:, :])
            nc.scalar.dma_start(out=m0_sb, in_=m0_d[:, :])
            nc.scalar.dma_start(out=bq_sb, in_=bq_d[:, :])
            nc.scalar.dma_start(out=bk_sb, in_=bk_d[:, :])
            nc.scalar.dma_start(out=bp_sb, in_=bp_d[:, :])

            qT = qkpool.tile([128, NCI, TP], bf16, tag="qT")
            kT = qkpool.tile([128, NCI, TP], bf16, tag="kT")
            vaug = vpool.tile([128, 7, VW], bf16, tag="vaug")
            yT = None

            def load_w(wdram, pfx, q):
                ws = []
                for ci in range(NCI):
                    wt = wpool.tile([128, C], bf16, tag=f"{pfx}{ci}",
                                    name=f"{pfx}{ci}")
                    q.dma_start(
                        out=wt, in_=wdram[ci * 128:(ci + 1) * 128, :])
                    ws.append(wt)
                return ws

            def proj_qk(w, x, b_sb, outT, psA, shpool):
                for cog in (range(0, 4), range(4, 8)):
                    pss = {}
                    for co in cog:
                        pss[co] = psA.tile([128, 1024], f32, tag="ps",
                                           name=f"psqk{co}")
                    for ci in range(NCI):
                        for co in cog:
                            for (lo, hi) in ((0, 512), (512, TP)):
                                nc.tensor.matmul(
                                    pss[co][:, lo:hi],
                                    w[ci][:, co * 128:(co + 1) * 128],
                                    x[:, ci, lo:hi],
                                    start=(ci == 0), stop=(ci == NCI - 1))
                    for co in cog:
                        proj_qk_tail(pss[co], co, b_sb, outT, shpool)

            def proj_qk_tail(ps, co, b_sb, outT, shpool):
                # bias add + PSUM->SBUF(bf16) on ACT
                nc.scalar.activation(
                    out=outT[:, co, :], in_=ps[:, 0:TP], func=Ident,
                    bias=b_sb[:, co:co + 1], scale=1.0)
                # rotary: swap 16-row blocks of the (host-permuted) rotary
                # dims, then q = q*cos + swapped*sin (cos rows outside the
                # rotary dims are 1.0, sin rows are 0.0).
                sh = shpool.tile([128, TP], bf16, tag="sh", name="sh")
                nc.sync.dma_start(
                    out=sh[32:64, :], in_=outT[32:64, co, :])
                for s in (0, 64):
                    nc.sync.dma_start(
                        out=sh[s:s + 16, :], in_=outT[s + 16:s + 32, co, :])
                    nc.sync.dma_start(
                        out=sh[s + 16:s + 32, :], in_=outT[s:s + 16, co, :])
                nc.vector.tensor_mul(
                    sh[0:96, :], sh[0:96, :], sin_sb[0:96, :])
                nc.vector.tensor_mul(
                    outT[:, co, :], outT[:, co, :], cos_sb)
                nc.vector.tensor_add(
                    outT[0:96, co, :], outT[0:96, co, :], sh[0:96, :])

            def proj_v(w, x, psA, bv_sb):
                for g in (range(0, 4), range(4, 7)):
                    pss = {}
                    for tt in g:
                        pss[tt] = psA.tile([128, 1024], f32, tag="ps",
                                           name=f"psv{tt}")
                    for ci in range(NCI):
                        for tt in g:
                            tsz = 128 if tt < 6 else 1
                            for hf in (0, 1):
                                nc.tensor.matmul(
                                    pss[tt][:tsz, hf * 512:hf * 512 + 512],
                                    x[:, ci, tt * 128:tt * 128 + tsz],
                                    w[ci][:, hf * 512:hf * 512 + 512],
                                    start=(ci == 0), stop=(ci == NCI - 1))
                    for tt in g:
                        tsz = 128 if tt < 6 else 1
                        va = vaug[:tsz, tt, :].rearrange(
                            "p (h e) -> p h e", e=HW)
                        nc.vector.tensor_add(
                            va[:, :, 0:HD],
                            pss[tt][:tsz, :].rearrange("p (h d) -> p h d", h=H),
                            bv_sb[:tsz, :].rearrange("p (h d) -> p h d", h=H))
                        nc.vector.tensor_copy(
                            va[:, :, HD:HD + 1], ones16[:tsz, :].unsqueeze(2))
                        nc.vector.tensor_copy(
                            va[:, :, HD + 1:HW], zcol[:tsz, 0:16].unsqueeze(2))

            def attn_j(j, psS, psO, pt_pool, rdbc_pool, dnd):
                """Head pair (2j, 2j+1): row-tiled S^T in R0/R1 psum chunks,
                fused 2-head exp per chunk, all S^T before all PV so a
                division-gated PV never dams the PE queue."""
                sts, pts = {}, {}
                for nk in range(7):
                    if nk < 6:
                        qlo, moff = R0SUB[nk]
                        ks = slice(nk * 128, (nk + 1) * 128)
                        st0 = psS.tile([128, 2, 512], f32, tag="stR0",
                                       name=f"stR0_{j}_{nk}")
                        pt0 = pt_pool.tile([128, 2, 512], bf16,
                                           tag=f"ptR0{nk}", bufs=1,
                                           name=f"ptR0_{j}_{nk}")
                        nc.tensor.matmul(
                            st0[:, 0, qlo:512], kT[0:64, j, ks],
                            qT[0:64, j, qlo:512], start=True, stop=True)
                        nc.tensor.matmul(
                            st0[:, 1, qlo:512], kT[64:128, j, ks],
                            qT[64:128, j, qlo:512], start=True, stop=True)
                        nc.scalar.activation(
                            out=pt0[:, :, qlo:512], in_=st0[:, :, qlo:512],
                            func=Exp, scale=0.125)
                        if moff is not None:
                            nc.gpsimd.tensor_mul(
                                pt0[:, 0, moff:moff + 128],
                                pt0[:, 0, moff:moff + 128], m0_sb)
                            nc.gpsimd.tensor_mul(
                                pt0[:, 1, moff:moff + 128],
                                pt0[:, 1, moff:moff + 128], m0_sb)
                        st1 = psS.tile([128, 2, 512], f32, tag="stR1",
                                       name=f"stR1_{j}_{nk}")
                        pt1 = pt_pool.tile([128, 2, 512], bf16,
                                           tag=f"ptR1{nk}", bufs=1,
                                           name=f"ptR1_{j}_{nk}")
                        nc.tensor.matmul(
                            st1[:, 0, 0:258], kT[0:64, j, ks],
                            qT[0:64, j, 512:TP], start=True, stop=True)
                        nc.tensor.matmul(
                            st1[:, 1, 0:258], kT[64:128, j, ks],
                            qT[64:128, j, 512:TP], start=True, stop=True)
                        nc.scalar.activation(
                            out=pt1[:, :, 0:258], in_=st1[:, :, 0:258],
                            func=Exp, scale=0.125)
                        sts[nk] = (st0, st1)
                        pts[nk] = (pt0, pt1)
                    else:
                        # kv row 768: q cols 512:770 only, col 512 masked
                        st1 = psS.tile([128, 2, 512], f32, tag="stR1",
                                       name=f"stR1_{j}_{nk}")
                        pt1 = pt_pool.tile([128, 2, 512], bf16,
                                           tag="ptR16", bufs=1,
                                           name=f"ptR1_{j}_{nk}")
                        nc.tensor.matmul(
                            st1[0:1, 0, 0:258], kT[0:64, j, 768:769],
                            qT[0:64, j, 512:TP], start=True, stop=True)
                        nc.tensor.matmul(
                            st1[0:1, 1, 0:258], kT[64:128, j, 768:769],
                            qT[64:128, j, 512:TP], start=True, stop=True)
                        nc.scalar.activation(
                            out=pt1[0:1, :, 0:258], in_=st1[0:1, :, 0:258],
                            func=Exp, scale=0.125)
                        nc.gpsimd.tensor_copy(
                            pt1[0:1, :, 0:1], zcol[0:1, 0:2].unsqueeze(2))
                        pts[nk] = (None, pt1)

                ov0 = psO.tile([128, 2, 512], f32, tag="ovR0", name=f"ov0_{j}")
                ov1 = psO.tile([128, 2, 512], f32, tag="ovR1", name=f"ov1_{j}")
                for nk in range(7):
                    pt0, pt1 = pts[nk]
                    qlo = R0SUB[nk][0] if nk < 6 else None
                    for h in (0, 1):
                        vs = slice((2 * j + h) * HW, (2 * j + h) * HW + HW)
                        if nk < 6:
                            nc.tensor.matmul(
                                ov0[0:HW, h, qlo:512], vaug[:, nk, vs],
                                pt0[:, h, qlo:512],
                                start=(nk == 0), stop=(nk == 5))
                            nc.tensor.matmul(
                                ov1[0:HW, h, 0:258], vaug[:, nk, vs],
                                pt1[:, h, 0:258],
                                start=(nk == 0), stop=False)
                        else:
                            nc.tensor.matmul(
                                ov1[0:HW, h, 0:258], vaug[0:1, 6, vs],
                                pt1[0:1, h, 0:258], start=False, stop=True)

                # softmax division: denominator rows (PSUM row 64) staged
                # to SBUF on gpsimd, to DRAM, one broadcast DMA back,
                # reciprocal, 4 mults.
                stg = rdbc_pool.tile([1, 2, TP], f32, tag="stg", bufs=2,
                                     name=f"stg{j}")
                nc.vector.tensor_copy(
                    stg[0:1, :, 0:512], ov0[HD:HD + 1, :, :])
                nc.sync.dma_start(
                    out=dnd[j:j + 1, :, 0:512], in_=stg[0:1, :, 0:512])
                nc.vector.tensor_copy(
                    stg[0:1, :, 512:TP], ov1[HD:HD + 1, :, 0:258])
                nc.sync.dma_start(
                    out=dnd[j:j + 1, :, 512:TP], in_=stg[0:1, :, 512:TP])
                rdbc = rdbc_pool.tile([128, TP], f32, tag="rdbc",
                                      name=f"rdbc{j}")
                nc.gpsimd.dma_start(
                    out=rdbc[0:64, :],
                    in_=dnd[j:j + 1, 0, :].broadcast_to((64, TP)))
                nc.gpsimd.dma_start(
                    out=rdbc[64:128, :],
                    in_=dnd[j:j + 1, 1, :].broadcast_to((64, TP)))
                nc.vector.reciprocal_approx_fast(out=rdbc, in_=rdbc)
                nc.vector.tensor_mul(
                    yT[0:64, j, 0:512], ov0[0:HD, 0, :], rdbc[0:64, 0:512])
                nc.vector.tensor_mul(
                    yT[64:128, j, 0:512], ov0[0:HD, 1, :], rdbc[64:128, 0:512])
                nc.vector.tensor_mul(
                    yT[0:64, j, 512:TP], ov1[0:HD, 0, 0:258],
                    rdbc[0:64, 512:TP])
                nc.vector.tensor_mul(
                    yT[64:128, j, 512:TP], ov1[0:HD, 1, 0:258],
                    rdbc[64:128, 512:TP])

            def proj_out(w, psB, opool):
                for cog in (range(0, 4), range(4, 8)):
                    pss = {}
                    for co in cog:
                        pss[co] = psB.tile([128, 1024], f32, tag="pso",
                                           name=f"pso{co}")
                    for ci in range(NCI):
                        for co in cog:
                            for (lo, hi) in ((0, 512), (512, TP)):
                                nc.tensor.matmul(
                                    pss[co][:, lo:hi],
                                    w[ci][:, co * 128:(co + 1) * 128],
                                    yT[:, ci, lo:hi],
                                    start=(ci == 0), stop=(ci == NCI - 1))
                    for co in cog:
                        ot = opool.tile([128, TP], bf16, tag="ot", name="ot")
                        nc.scalar.activation(
                            out=ot, in_=pss[co][:, 0:TP], func=Ident,
                            bias=bp_sb[:, co:co + 1], scale=1.0)
                        nc.sync.dma_start(
                            out=out_d[co * 128:(co + 1) * 128, :], in_=ot)

            # ---- phase 1: projections ----
            with (
                tc.tile_pool(name="psA", bufs=4, space="PSUM") as psA,
                tc.tile_pool(name="xq", bufs=1) as xqp,
                tc.tile_pool(name="xkv", bufs=1) as xkp,
                tc.tile_pool(name="shpool", bufs=2) as shpool,
            ):
                bv_sb = xqp.tile([128, C], f32, tag="bv")
                nc.gpsimd.dma_start(
                    out=bv_sb, in_=bv_d[0:1, :].broadcast_to((128, C)))
                xq = xqp.tile([128, NCI, TP], bf16, tag="xq")
                xkv = xkp.tile([128, NCI, TP], bf16, tag="xkv")
                # sync queue: xq+wq interleaved per-ci so MMs start early;
                # scalar queue: xkv+wk in parallel; tensor queue: wv, wp.
                wq, wk = [], []
                for ci in range(NCI):
                    nc.sync.dma_start(
                        out=xq[:, ci, :],
                        in_=xqT_d[ci * 128:(ci + 1) * 128, :])
                    wt = wpool.tile([128, C], bf16, tag=f"wq{ci}",
                                    name=f"wq{ci}")
                    nc.sync.dma_start(
                        out=wt, in_=wq_d[ci * 128:(ci + 1) * 128, :])
                    wq.append(wt)
                    nc.scalar.dma_start(
                        out=xkv[:, ci, :],
                        in_=xkvT_d[ci * 128:(ci + 1) * 128, :])
                wk = load_w(wk_d, "wk", nc.scalar)
                wv = load_w(wv_d, "wv", nc.gpsimd)
                proj_qk(wq, xq, bq_sb, qT, psA, shpool)
                proj_qk(wk, xkv, bk_sb, kT, psA, shpool)
                proj_v(wv, xkv, psA, bv_sb)

            # ---- phases 2+3 share yT ----
            with tc.tile_pool(name="ypool", bufs=1) as ypool:
                yT = ypool.tile([128, NCI, TP], bf16, tag="yT")
                # ---- phase 2: attention ----
                with (
                    tc.tile_pool(name="psS", bufs=1, space="PSUM") as psS,
                    tc.tile_pool(name="psO", bufs=1, space="PSUM") as psO,
                    tc.tile_pool(name="ptp", bufs=1) as pt_pool,
                    tc.tile_pool(name="rdbcp", bufs=2) as rdbc_pool,
                    tc.tile_pool(name="rddp", bufs=1, space="DRAM") as dram_pool,
                ):
                    wp = load_w(wp_d, "wp", nc.gpsimd)  # prefetch
                    dnd = dram_pool.tile([NCI, 2, TP], f32, tag="dnd")
                    for j in range(NCI):
                        attn_j(j, psS, psO, pt_pool, rdbc_pool, dnd)

                # ---- phase 3: output projection ----
                with (
                    tc.tile_pool(name="psB", bufs=4, space="PSUM") as psB,
                    tc.tile_pool(name="opool", bufs=3) as opool,
                ):
                    proj_out(wp, psB, opool)

    nc.compile()
    return nc


def _host_prep(x_q, x_kv, rotary_pos_emb, Wq, bq, Wk, bk, Wv, bv, Wp, bp):
    f = np.float32
    x_q = np.asarray(x_q, f)
    x_kv = np.asarray(x_kv, f)
    freqs = np.asarray(rotary_pos_emb, f)

    # Even/odd pair-split permutation of the first 32 dims of each head, so
    # rotate_half becomes a 16-partition block swap on chip.
    perm = np.arange(C)
    for h in range(H):
        b0 = h * HD
        blk = np.empty(HD, np.int64)
        blk[0:16] = b0 + np.arange(0, 32, 2)
        blk[16:32] = b0 + np.arange(1, 32, 2)
        blk[32:64] = b0 + np.arange(32, 64)
        perm[b0:b0 + HD] = blk

    def wT(W, p=None):
        W = np.asarray(W, f)
        if p is not None:
            W = W[p, :]
        return np.ascontiguousarray(W.T).astype(np.float32)

    cosE = np.cos(freqs[:, 0::2]).T
    cosO = np.cos(freqs[:, 1::2]).T
    sinE = -np.sin(freqs[:, 0::2]).T
    sinO = np.sin(freqs[:, 1::2]).T
    cosP = np.ones((128, TP), f)
    sinP = np.zeros((128, TP), f)
    for s in (0, 64):
        cosP[s:s + 16, :T] = cosE
        cosP[s + 16:s + 32, :T] = cosO
        sinP[s:s + 16, :T] = sinE
        sinP[s + 16:s + 32, :T] = sinO

    p_idx = np.arange(128)[:, None]
    f_idx = np.arange(128)[None, :]
    m0 = (p_idx < f_idx).astype(f)

    import ml_dtypes
    bf = ml_dtypes.bfloat16

    bqp = np.asarray(bq, f)[perm]
    bkp = np.asarray(bk, f)[perm]
    shared = {
        "wqT": wT(Wq, perm).astype(bf),
        "wkT": wT(Wk, perm).astype(bf),
        "wvT": wT(Wv).astype(bf),
        "wpT": wT(Wp).astype(bf),
        "bq2": np.ascontiguousarray(bqp.reshape(NCO, 128).T).astype(f),
        "bk2": np.ascontiguousarray(bkp.reshape(NCO, 128).T).astype(f),
        "bp2": np.ascontiguousarray(
            np.asarray(bp, f).reshape(NCO, 128).T).astype(f),
        "bv1": np.asarray(bv, f).reshape(1, C).copy(),
        "cosP": np.ascontiguousarray(cosP).astype(bf),
        "sinP": np.ascontiguousarray(sinP).astype(bf),
        "m0": np.ascontiguousarray(m0).astype(bf),
    }

    def padT(xt):
        out = np.zeros((C, TP), np.float32)
        out[:, :T] = xt
        return out.astype(bf)

    in_maps = []
    for b in range(B):
        m = dict(shared)
        m["xqT"] = padT(x_q[b].T)
        m["xkvT"] = padT(x_kv[b].T)
        in_maps.append(m)
    return in_maps


def kernel(x_q, x_kv, rotary_pos_emb, Wq, bq, Wk, bk, Wv, bv, Wp, bp):
    from concourse.bass_utils import run_bass_kernel_spmd

    if "nc" not in _CACHE:
        _CACHE["nc"] = _build_program()
    nc = _CACHE["nc"]

    in_maps = _host_prep(x_q, x_kv, rotary_pos_emb,
                         Wq, bq, Wk, bk, Wv, bv, Wp, bp)
    trace = os.environ.get("BTK_TRACE", "0") == "1"
    res = run_bass_kernel_spmd(
        nc, in_maps, core_ids=list(range(B)), trace=trace)
    _CACHE["last_result"] = res
    out = np.stack(
        [np.asarray(r["outT"][:, :T], np.float32).T for r in res.results],
        axis=0)
    return out
